# revision 15
# baseline (speedup 1.0000x reference)
"""Bahdanau additive attention on 8 Trainium2 NeuronCores.

  score_t = V^T tanh(W1 value_t + W2 query);  out = softmax(score) @ value

Sharding: data-parallel over batch (16 batches -> 2 per core), weights
replicated.

Design: a per-supertile (1024 t) software pipeline, all bf16 compute
against fp32 PSUM accumulation:
  load fp32 (8 KiB lines, p-major)                   [DMA, sync ring]
  -> cast bf16                                       [DVE]
  -> xbar DMA-transpose (value^T blocks)             [DMA, sync ring]
  -> keys^T = W1^T @ value^T per 512-chunk           [PE, 4-deep PSUM]
  -> tanh(psK + W2 q bias) per chunk                 [ACT]
  -> score rows = V^T th                             [PE]
  -> score row evacuation PSUM->SBUF                 [ACT/DVE]
  -> per-4-supertile-group scatter -> [32,128]       [DMA, sync ring]
  -> PE transpose -> exp -> e^T bf16                 [PE, ACT]
  -> context += e^T_f * value_f  (bf16)              [PE, streaming PSUM]
Stage lags keep the sync ring's descriptor generator (the xbar's real
serializing cost) fed with loads ahead of every transpose, and PSUM is
chunked [128,512] x4 so keys/tanh ping-pong never stalls the array.
"""

import functools
import os
import sys

import numpy as np

if "/opt/trn_rl_repo" not in sys.path:
    sys.path.insert(0, "/opt/trn_rl_repo")

B, T, D, U = 16, 8192, 256, 256
NCORES = 8
BPC = B // NCORES          # batches per core
P = 128                    # SBUF partitions
ST = 1024                  # t per supertile
NST = T // ST              # supertiles per batch
NSLOT = BPC * NST          # pipeline slots per core
CH = 512                   # score/keys chunk width (PSUM bank = 512 fp32)
NCH = ST // CH             # chunks per supertile
FPB = ST // P              # f rows per supertile (8)
GRP = 4                    # supertiles per score-transpose group


@functools.lru_cache(maxsize=1)
def _build():
    from contextlib import ExitStack

    import concourse.bass as bass
    import concourse.tile as tile
    from concourse import bacc, mybir
    from concourse.masks import make_identity

    f32 = mybir.dt.float32
    bf16 = mybir.dt.bfloat16
    Act = mybir.ActivationFunctionType

    nc = bacc.Bacc("TRN2", target_bir_lowering=False, debug=False)

    q = nc.dram_tensor("query", [BPC, D], f32, kind="ExternalInput").ap()
    val = nc.dram_tensor("value", [BPC, T, D], f32, kind="ExternalInput").ap()
    w1 = nc.dram_tensor("W1", [D, U], f32, kind="ExternalInput").ap()
    w2 = nc.dram_tensor("W2", [D, U], f32, kind="ExternalInput").ap()
    vv = nc.dram_tensor("V", [U, 1], f32, kind="ExternalInput").ap()
    out = nc.dram_tensor("out", [BPC, D], f32, kind="ExternalOutput").ap()

    with tile.TileContext(nc) as tc, ExitStack() as ctx:
        consts = ctx.enter_context(tc.tile_pool(name="consts", bufs=1))
        v32pool = ctx.enter_context(tc.tile_pool(name="v32", bufs=6))
        vbpool = ctx.enter_context(tc.tile_pool(name="vb", bufs=11))
        vtpool = ctx.enter_context(tc.tile_pool(name="vt", bufs=4))
        thpool = ctx.enter_context(tc.tile_pool(name="th", bufs=2))
        scpool = ctx.enter_context(tc.tile_pool(name="scrow", bufs=2))
        s8pool = ctx.enter_context(tc.tile_pool(name="s8", bufs=2))
        psk = ctx.enter_context(tc.tile_pool(name="psk", bufs=4, space="PSUM"))
        pssc = ctx.enter_context(tc.tile_pool(name="pssc", bufs=2, space="PSUM"))
        pst = ctx.enter_context(tc.tile_pool(name="pst", bufs=1, space="PSUM"))
        psc = ctx.enter_context(tc.tile_pool(name="psc", bufs=1, space="PSUM"))

        # ---- constants / weights (DMA on the gpsimd ring so value loads
        # own the sync ring from the first instruction) -------------------
        ident = consts.tile([64, 64], f32)
        make_identity(nc, ident)

        ones = consts.tile([P, 1], f32)
        nc.gpsimd.memset(ones, 1.0)

        # sel4 = [[1,0],[1,0],[0,1],[0,1]] — engine partition ranges must be
        # 32-aligned, so rows 2-3 are set via tiny DMAs from the identity.
        sel4 = consts.tile([4, 2], f32)
        nc.gpsimd.memset(sel4, 0.0)
        nc.gpsimd.memset(sel4[0:2, 0:1], 1.0)
        for r in (2, 3):
            nc.gpsimd.dma_start(out=sel4[r:r + 1, 1:2], in_=ident[0:1, 0:1])

        # W1 in d-half layout (d = kb*128 + p), bf16
        w1f = consts.tile([P, 2, U], f32)
        nc.gpsimd.dma_start(
            out=w1f, in_=w1.rearrange("(kb p) u -> p kb u", p=P)
        )
        w1b = consts.tile([P, 2, U], bf16)
        nc.vector.tensor_copy(out=w1b, in_=w1f)
        # V in u-half layout (u = ub*128 + p), bf16
        vvf = consts.tile([P, 2, 1], f32)
        nc.gpsimd.dma_start(out=vvf, in_=vv.rearrange("(ub p) o -> p ub o", p=P))
        vsb = consts.tile([P, 2, 1], bf16)
        nc.vector.tensor_copy(out=vsb, in_=vvf)

        w2b = consts.tile([P, 2, U], f32)
        nc.gpsimd.dma_start(out=w2b, in_=w2.rearrange("(kb p) u -> p kb u", p=P))

        # hidden = query @ W2, computed as hidden^T [u, b] so it can feed
        # the tanh as a per-partition bias.
        q_sb = consts.tile([BPC, D], f32)
        nc.gpsimd.dma_start(out=q_sb, in_=q)
        qt = consts.tile([P, 2, BPC], f32)
        for kb in range(2):
            psq = pssc.tile([P, CH], f32, tag="pssc", name="psq")
            nc.tensor.transpose(
                out=psq[:, 0:BPC],
                in_=q_sb[:, P * kb:P * (kb + 1)],
                identity=ident[0:BPC, 0:BPC],
            )
            nc.scalar.copy(out=qt[:, kb, :], in_=psq[:, 0:BPC])
        hid = []
        for u in range(2):
            psh = pssc.tile([P, CH], f32, tag="pssc", name="psh")
            for kb in range(2):
                nc.tensor.matmul(
                    psh[:, 0:BPC],
                    lhsT=w2b[:, kb, P * u:P * (u + 1)],
                    rhs=qt[:, kb, :],
                    start=(kb == 0),
                    stop=(kb == 1),
                )
            h = consts.tile([P, BPC], f32, tag=f"hid{u}")
            nc.scalar.copy(out=h, in_=psh[:, 0:BPC])
            hid.append(h)

        eT = [
            consts.tile([P, FPB * NST], bf16, tag=f"eT{b}", name=f"eT{b}")
            for b in range(BPC)
        ]

        # persistent context accumulator: rows {0,32} = batch 0 (f even/odd),
        # rows {64,96} = batch 1.
        psC = psc.tile([P, CH], f32, tag="psc", name="psc")

        vb32s, vbs, vts = {}, {}, {}
        scrows, s8s = {}, {}

        def emit_load(s):
            b, j = divmod(s, NST)
            VB32 = v32pool.tile([P, FPB, D], f32, tag="vb32", name="vb32")
            vb32s[s] = VB32
            nc.sync.dma_start(
                out=VB32,
                in_=val[b, ST * j:ST * (j + 1), :].rearrange(
                    "(p f) d -> p f d", f=FPB
                ),
            )

        def emit_casts(s):
            VB32 = vb32s[s]
            VB = vbpool.tile([P, 2, FPB, P], bf16, tag="vb", name="vb")
            vbs[s] = VB
            nc.vector.tensor_copy(
                out=VB, in_=VB32.rearrange("p f (h d) -> p h f d", h=2)
            )

        def emit_xbar(s):
            VB = vbs[s]
            VT = vtpool.tile([P, 2, FPB, P], bf16, tag="vt", name="vt")
            vts[s] = VT
            nc.sync.dma_start(
                out=VT.rearrange("p h f t -> p (h f) t"),
                in_=VB.rearrange("p h f t -> p (h f t)"),
                transpose=True,
            )

        def emit_mid(s):
            b, j = divmod(s, NST)
            VT = vts[s]
            th = thpool.tile([P, 2, ST], bf16, tag="th", name="th")
            for uh in range(2):
                psKc = []
                for c in range(NCH):
                    psKc.append(psk.tile([P, CH], f32, tag="psk", name="psk"))
                for dk in range(2):
                    for c in range(NCH):
                        nc.tensor.matmul(
                            psKc[c],
                            lhsT=w1b[:, dk, P * uh:P * (uh + 1)],
                            rhs=VT[:, dk, 4 * c:4 * (c + 1), :].rearrange(
                                "p f t -> p (f t)"
                            ),
                            start=(dk == 0),
                            stop=(dk == 1),
                        )
                for c in range(NCH):
                    nc.scalar.activation(
                        out=th[:, uh, CH * c:CH * (c + 1)],
                        in_=psKc[c],
                        func=Act.Tanh,
                        bias=hid[uh][:, b:b + 1],
                        scale=1.0,
                    )
            psSC = pssc.tile([P, CH], f32, tag="pssc", name="pssc")
            for c in range(NCH):
                for uh in range(2):
                    nc.tensor.matmul(
                        psSC[32 * c:32 * c + 1, :],
                        lhsT=vsb[:, uh, :],
                        rhs=th[:, uh, CH * c:CH * (c + 1)],
                        start=(uh == 0),
                        stop=(uh == 1),
                        tile_position=(0, 32 * c),
                    )
            g, r = divmod(s, GRP)
            if r == 0:
                scrow = scpool.tile(
                    [P, GRP * CH], f32, tag="scrow", name="scrow"
                )
                scrows[g] = scrow
            scrow = scrows[g]
            nc.scalar.copy(
                out=scrow[0:1, CH * r:CH * (r + 1)], in_=psSC[0:1, :]
            )
            nc.vector.tensor_copy(
                out=scrow[32:33, CH * r:CH * (r + 1)], in_=psSC[32:33, :]
            )

        def emit_scatter(g):
            scrow = scrows[g]
            s8 = s8pool.tile([GRP * FPB, P], f32, tag="s8", name="s8")
            s8s[g] = s8
            for r in range(GRP):
                for c in range(NCH):
                    row = FPB * r + 4 * c
                    nc.sync.dma_start(
                        out=s8[row:row + 4, :],
                        in_=scrow[
                            32 * c:32 * c + 1, CH * r:CH * (r + 1)
                        ].rearrange("o (k f) -> o k f", k=4),
                    )

        def emit_tail(g):
            # handles slots GRP*g .. GRP*g+3 (all within one batch)
            s8 = s8s[g]
            psT = pst.tile([P, GRP * FPB], f32, tag="pst", name="pst")
            nc.tensor.transpose(
                out=psT, in_=s8, identity=ident[0:GRP * FPB, 0:GRP * FPB]
            )
            b = (GRP * g) // NST
            gj = (GRP * g) % NST          # first supertile of the group
            nc.scalar.activation(
                out=eT[b][:, FPB * gj:FPB * (gj + GRP)],
                in_=psT,
                func=Act.Exp,
                scale=1.0,
            )
            for r in range(GRP):
                s = GRP * g + r
                j = gj + r
                VB = vbs[s]
                for f in range(FPB):
                    par = f % 2
                    row = 32 * (2 * b + par)
                    nc.tensor.matmul(
                        psC[row:row + 1, 0:D],
                        lhsT=eT[b][:, FPB * j + f:FPB * j + f + 1],
                        rhs=VB[:, :, f, :],
                        start=(j == 0 and f == par),
                        stop=(j == NST - 1 and f == FPB - 2 + par),
                        tile_position=(0, row),
                    )

        for s in range(NSLOT + 7):
            if s < NSLOT:
                emit_load(s)
            if 0 <= s - 1 < NSLOT:
                emit_casts(s - 1)
            if 0 <= s - 2 < NSLOT:
                emit_xbar(s - 2)
            if 0 <= s - 3 < NSLOT:
                emit_mid(s - 3)
            if s >= 5 and (s - 5) % GRP == GRP - 1 and (s - 5) < NSLOT:
                emit_scatter((s - 5) // GRP)
            if s >= 6 and (s - 6) % GRP == GRP - 1 and (s - 6) < NSLOT:
                emit_tail((s - 6) // GRP)

        # ---- final combine: fold parity rows, normalize, store ----------
        # softmax normalizer S_b = sum(e): eT free-reduce then a ones-matmul
        # into one PSUM row; evacuated next to the context rows so the sel4
        # fold produces [ctx_b | S_b] per batch in one matmul.
        psS = pst.tile([P, GRP * FPB], f32, tag="pst", name="psS")
        for b in range(BPC):
            esum = consts.tile([P, 1], f32, tag=f"esum{b}", name=f"esum{b}")
            nc.vector.reduce_sum(out=esum, in_=eT[b], axis=mybir.AxisListType.X)
            nc.tensor.matmul(
                psS[0:1, b:b + 1], lhsT=ones, rhs=esum, start=True, stop=True
            )
        ssb = consts.tile([1, BPC], f32)
        nc.scalar.copy(out=ssb, in_=psS[0:1, 0:BPC])
        cpstage = consts.tile([P, D], f32)
        for r in (0, 32, 64, 96):
            nc.scalar.copy(out=cpstage[r:r + 1, :], in_=psC[r:r + 1, 0:D])
        cp4 = consts.tile([4, D + 1], f32)
        for k in range(4):
            nc.gpsimd.dma_start(
                out=cp4[k:k + 1, 0:D], in_=cpstage[32 * k:32 * k + 1, :]
            )
        # normalizer column: S_b lands at cp4[2b, D] (other rows zeroed)
        nc.gpsimd.memset(cp4[:, D:D + 1], 0.0)
        for b in range(BPC):
            nc.gpsimd.dma_start(
                out=cp4[2 * b:2 * b + 1, D:D + 1], in_=ssb[0:1, b:b + 1]
            )
        ps2 = pssc.tile([P, CH], f32, tag="pssc", name="ps2")
        nc.tensor.matmul(
            ps2[0:BPC, 0:D + 1], lhsT=sel4, rhs=cp4, start=True, stop=True
        )
        inv2 = consts.tile([BPC, 1], f32)
        nc.vector.reciprocal(out=inv2, in_=ps2[0:BPC, D:D + 1])
        ctxout = consts.tile([BPC, D], f32)
        nc.vector.tensor_scalar_mul(
            out=ctxout, in0=ps2[0:BPC, 0:D], scalar1=inv2
        )
        nc.sync.dma_start(out=out, in_=ctxout)

    nc.finalize()
    return nc


def _run(inputs, trace=False):
    from concourse import bass_utils

    nc = _build()
    in_maps = [
        {
            "query": np.ascontiguousarray(inputs["query"][BPC * i:BPC * (i + 1)]),
            "value": np.ascontiguousarray(inputs["value"][BPC * i:BPC * (i + 1)]),
            "W1": np.asarray(inputs["W1"]),
            "W2": np.asarray(inputs["W2"]),
            "V": np.asarray(inputs["V"]),
        }
        for i in range(NCORES)
    ]
    res = bass_utils.run_bass_kernel_spmd(
        nc, in_maps, core_ids=list(range(NCORES)), trace=trace
    )
    outp = np.concatenate([r["out"] for r in res.results], axis=0)
    return outp.astype(np.float32), res


def kernel(**inputs) -> np.ndarray:
    outp, _ = _run(inputs, trace=False)
    return outp


# revision 16
# speedup vs baseline: 1.0433x; 1.0433x over previous
"""Bahdanau additive attention on 8 Trainium2 NeuronCores.

  score_t = V^T tanh(W1 value_t + W2 query);  out = softmax(score) @ value

Sharding: data-parallel over batch (16 batches -> 2 per core), weights
replicated.

Design: a per-supertile (1024 t) software pipeline, all bf16 compute
against fp32 PSUM accumulation:
  load fp32 (8 KiB lines, p-major)                   [DMA, sync ring]
  -> cast bf16                                       [DVE]
  -> xbar DMA-transpose (value^T blocks)             [DMA, sync ring]
  -> keys^T = W1^T @ value^T per 512-chunk           [PE, 4-deep PSUM]
  -> tanh(psK + W2 q bias) per chunk                 [ACT]
  -> score rows = V^T th                             [PE]
  -> score row evacuation PSUM->SBUF                 [ACT/DVE]
  -> per-4-supertile-group scatter -> [32,128]       [DMA, sync ring]
  -> PE transpose -> exp -> e^T bf16                 [PE, ACT]
  -> context += e^T_f * value_f  (bf16)              [PE, streaming PSUM]
Stage lags keep the sync ring's descriptor generator (the xbar's real
serializing cost) fed with loads ahead of every transpose, and PSUM is
chunked [128,512] x4 so keys/tanh ping-pong never stalls the array.
"""

import functools
import os
import sys

import numpy as np

if "/opt/trn_rl_repo" not in sys.path:
    sys.path.insert(0, "/opt/trn_rl_repo")

B, T, D, U = 16, 8192, 256, 256
NCORES = 8
BPC = B // NCORES          # batches per core
P = 128                    # SBUF partitions
ST = 1024                  # t per supertile
NST = T // ST              # supertiles per batch
NSLOT = BPC * NST          # pipeline slots per core
CH = 512                   # score/keys chunk width (PSUM bank = 512 fp32)
NCH = ST // CH             # chunks per supertile
FPB = ST // P              # f rows per supertile (8)
GRP = 4                    # supertiles per score-transpose group


@functools.lru_cache(maxsize=1)
def _build():
    from contextlib import ExitStack

    import concourse.bass as bass
    import concourse.tile as tile
    from concourse import bacc, mybir
    from concourse.masks import make_identity

    f32 = mybir.dt.float32
    bf16 = mybir.dt.bfloat16
    Act = mybir.ActivationFunctionType

    nc = bacc.Bacc("TRN2", target_bir_lowering=False, debug=False)

    q = nc.dram_tensor("query", [BPC, D], f32, kind="ExternalInput").ap()
    val = nc.dram_tensor("value", [BPC, T, D], f32, kind="ExternalInput").ap()
    w1 = nc.dram_tensor("W1", [D, U], f32, kind="ExternalInput").ap()
    w2 = nc.dram_tensor("W2", [D, U], f32, kind="ExternalInput").ap()
    vv = nc.dram_tensor("V", [U, 1], f32, kind="ExternalInput").ap()
    out = nc.dram_tensor("out", [BPC, D], f32, kind="ExternalOutput").ap()

    with tile.TileContext(nc) as tc, ExitStack() as ctx:
        consts = ctx.enter_context(tc.tile_pool(name="consts", bufs=1))
        v32pool = ctx.enter_context(tc.tile_pool(name="v32", bufs=8))
        vbpool = ctx.enter_context(tc.tile_pool(name="vb", bufs=12))
        vtpool = ctx.enter_context(tc.tile_pool(name="vt", bufs=8))
        thpool = ctx.enter_context(tc.tile_pool(name="th", bufs=3))
        scpool = ctx.enter_context(tc.tile_pool(name="scrow", bufs=2))
        s8pool = ctx.enter_context(tc.tile_pool(name="s8", bufs=2))
        psk = ctx.enter_context(tc.tile_pool(name="psk", bufs=4, space="PSUM"))
        pssc = ctx.enter_context(tc.tile_pool(name="pssc", bufs=2, space="PSUM"))
        pst = ctx.enter_context(tc.tile_pool(name="pst", bufs=1, space="PSUM"))
        psc = ctx.enter_context(tc.tile_pool(name="psc", bufs=1, space="PSUM"))

        # ---- constants / weights (DMA on the gpsimd ring so value loads
        # own the sync ring from the first instruction) -------------------
        ident = consts.tile([64, 64], f32)
        make_identity(nc, ident)

        ones = consts.tile([P, 1], f32)
        nc.gpsimd.memset(ones, 1.0)

        # sel4 = [[1,0],[1,0],[0,1],[0,1]] — engine partition ranges must be
        # 32-aligned, so rows 2-3 are set via tiny DMAs from the identity.
        sel4 = consts.tile([4, 2], f32)
        nc.gpsimd.memset(sel4, 0.0)
        nc.gpsimd.memset(sel4[0:2, 0:1], 1.0)
        for r in (2, 3):
            nc.gpsimd.dma_start(out=sel4[r:r + 1, 1:2], in_=ident[0:1, 0:1])

        # W1 in d-half layout (d = kb*128 + p), bf16
        w1f = consts.tile([P, 2, U], f32)
        nc.gpsimd.dma_start(
            out=w1f, in_=w1.rearrange("(kb p) u -> p kb u", p=P)
        )
        w1b = consts.tile([P, 2, U], bf16)
        nc.vector.tensor_copy(out=w1b, in_=w1f)
        # V in u-half layout (u = ub*128 + p), bf16
        vvf = consts.tile([P, 2, 1], f32)
        nc.gpsimd.dma_start(out=vvf, in_=vv.rearrange("(ub p) o -> p ub o", p=P))
        vsb = consts.tile([P, 2, 1], bf16)
        nc.vector.tensor_copy(out=vsb, in_=vvf)

        w2b = consts.tile([P, 2, U], f32)
        nc.gpsimd.dma_start(out=w2b, in_=w2.rearrange("(kb p) u -> p kb u", p=P))

        # hidden = query @ W2, computed as hidden^T [u, b] so it can feed
        # the tanh as a per-partition bias.
        q_sb = consts.tile([BPC, D], f32)
        nc.gpsimd.dma_start(out=q_sb, in_=q)
        qt = consts.tile([P, 2, BPC], f32)
        for kb in range(2):
            psq = pssc.tile([P, CH], f32, tag="pssc", name="psq")
            nc.tensor.transpose(
                out=psq[:, 0:BPC],
                in_=q_sb[:, P * kb:P * (kb + 1)],
                identity=ident[0:BPC, 0:BPC],
            )
            nc.scalar.copy(out=qt[:, kb, :], in_=psq[:, 0:BPC])
        hid = []
        for u in range(2):
            psh = pssc.tile([P, CH], f32, tag="pssc", name="psh")
            for kb in range(2):
                nc.tensor.matmul(
                    psh[:, 0:BPC],
                    lhsT=w2b[:, kb, P * u:P * (u + 1)],
                    rhs=qt[:, kb, :],
                    start=(kb == 0),
                    stop=(kb == 1),
                )
            h = consts.tile([P, BPC], f32, tag=f"hid{u}")
            nc.scalar.copy(out=h, in_=psh[:, 0:BPC])
            hid.append(h)

        eT = [
            consts.tile([P, FPB * NST], bf16, tag=f"eT{b}", name=f"eT{b}")
            for b in range(BPC)
        ]

        # persistent context accumulator: rows {0,32} = batch 0 (f even/odd),
        # rows {64,96} = batch 1.
        psC = psc.tile([P, CH], f32, tag="psc", name="psc")

        vb32s, vbs, vts = {}, {}, {}
        scrows, s8s = {}, {}

        def emit_load(s):
            b, j = divmod(s, NST)
            VB32 = v32pool.tile([P, FPB, D], f32, tag="vb32", name="vb32")
            vb32s[s] = VB32
            nc.sync.dma_start(
                out=VB32,
                in_=val[b, ST * j:ST * (j + 1), :].rearrange(
                    "(p f) d -> p f d", f=FPB
                ),
            )

        def emit_casts(s):
            VB32 = vb32s[s]
            VB = vbpool.tile([P, 2, FPB, P], bf16, tag="vb", name="vb")
            vbs[s] = VB
            nc.vector.tensor_copy(
                out=VB, in_=VB32.rearrange("p f (h d) -> p h f d", h=2)
            )

        def emit_xbar(s):
            VB = vbs[s]
            VT = vtpool.tile([P, 2, FPB, P], bf16, tag="vt", name="vt")
            vts[s] = VT
            nc.sync.dma_start(
                out=VT.rearrange("p h f t -> p (h f) t"),
                in_=VB.rearrange("p h f t -> p (h f t)"),
                transpose=True,
            )

        def emit_mid(s):
            b, j = divmod(s, NST)
            VT = vts[s]
            th = thpool.tile([P, 2, ST], bf16, tag="th", name="th")
            for uh in range(2):
                psKc = []
                for c in range(NCH):
                    psKc.append(psk.tile([P, CH], f32, tag="psk", name="psk"))
                for dk in range(2):
                    for c in range(NCH):
                        nc.tensor.matmul(
                            psKc[c],
                            lhsT=w1b[:, dk, P * uh:P * (uh + 1)],
                            rhs=VT[:, dk, 4 * c:4 * (c + 1), :].rearrange(
                                "p f t -> p (f t)"
                            ),
                            start=(dk == 0),
                            stop=(dk == 1),
                        )
                for c in range(NCH):
                    nc.scalar.activation(
                        out=th[:, uh, CH * c:CH * (c + 1)],
                        in_=psKc[c],
                        func=Act.Tanh,
                        bias=hid[uh][:, b:b + 1],
                        scale=1.0,
                    )
            psSC = pssc.tile([P, CH], f32, tag="pssc", name="pssc")
            for c in range(NCH):
                for uh in range(2):
                    nc.tensor.matmul(
                        psSC[32 * c:32 * c + 1, :],
                        lhsT=vsb[:, uh, :],
                        rhs=th[:, uh, CH * c:CH * (c + 1)],
                        start=(uh == 0),
                        stop=(uh == 1),
                        tile_position=(0, 32 * c),
                    )
            g, r = divmod(s, GRP)
            if r == 0:
                scrow = scpool.tile(
                    [P, GRP * CH], f32, tag="scrow", name="scrow"
                )
                scrows[g] = scrow
            scrow = scrows[g]
            nc.scalar.copy(
                out=scrow[0:1, CH * r:CH * (r + 1)], in_=psSC[0:1, :]
            )
            nc.vector.tensor_copy(
                out=scrow[32:33, CH * r:CH * (r + 1)], in_=psSC[32:33, :]
            )

        def emit_scatter(g):
            scrow = scrows[g]
            s8 = s8pool.tile([GRP * FPB, P], f32, tag="s8", name="s8")
            s8s[g] = s8
            for r in range(GRP):
                for c in range(NCH):
                    row = FPB * r + 4 * c
                    nc.sync.dma_start(
                        out=s8[row:row + 4, :],
                        in_=scrow[
                            32 * c:32 * c + 1, CH * r:CH * (r + 1)
                        ].rearrange("o (k f) -> o k f", k=4),
                    )

        def emit_tail(g):
            # handles slots GRP*g .. GRP*g+3 (all within one batch)
            s8 = s8s[g]
            psT = pst.tile([P, GRP * FPB], f32, tag="pst", name="pst")
            nc.tensor.transpose(
                out=psT, in_=s8, identity=ident[0:GRP * FPB, 0:GRP * FPB]
            )
            b = (GRP * g) // NST
            gj = (GRP * g) % NST          # first supertile of the group
            nc.scalar.activation(
                out=eT[b][:, FPB * gj:FPB * (gj + GRP)],
                in_=psT,
                func=Act.Exp,
                scale=1.0,
            )
            for r in range(GRP):
                s = GRP * g + r
                j = gj + r
                VB = vbs[s]
                for f in range(FPB):
                    par = f % 2
                    row = 32 * (2 * b + par)
                    nc.tensor.matmul(
                        psC[row:row + 1, 0:D],
                        lhsT=eT[b][:, FPB * j + f:FPB * j + f + 1],
                        rhs=VB[:, :, f, :],
                        start=(j == 0 and f == par),
                        stop=(j == NST - 1 and f == FPB - 2 + par),
                        tile_position=(0, row),
                    )

        for s in range(NSLOT + 8):
            if s < NSLOT:
                emit_load(s)
            if 0 <= s - 1 < NSLOT:
                emit_casts(s - 1)
            if 0 <= s - 2 < NSLOT:
                emit_xbar(s - 2)
            if 0 <= s - 4 < NSLOT:
                emit_mid(s - 4)
            if s >= 6 and (s - 6) % GRP == GRP - 1 and (s - 6) < NSLOT:
                emit_scatter((s - 6) // GRP)
            if s >= 7 and (s - 7) % GRP == GRP - 1 and (s - 7) < NSLOT:
                emit_tail((s - 7) // GRP)

        # ---- final combine: fold parity rows, normalize, store ----------
        # softmax normalizer S_b = sum(e): eT free-reduce then a ones-matmul
        # into one PSUM row; evacuated next to the context rows so the sel4
        # fold produces [ctx_b | S_b] per batch in one matmul.
        psS = pst.tile([P, GRP * FPB], f32, tag="pst", name="psS")
        for b in range(BPC):
            esum = consts.tile([P, 1], f32, tag=f"esum{b}", name=f"esum{b}")
            nc.vector.reduce_sum(out=esum, in_=eT[b], axis=mybir.AxisListType.X)
            nc.tensor.matmul(
                psS[0:1, b:b + 1], lhsT=ones, rhs=esum, start=True, stop=True
            )
        ssb = consts.tile([1, BPC], f32)
        nc.scalar.copy(out=ssb, in_=psS[0:1, 0:BPC])
        cpstage = consts.tile([P, D], f32)
        for r in (0, 32, 64, 96):
            nc.scalar.copy(out=cpstage[r:r + 1, :], in_=psC[r:r + 1, 0:D])
        cp4 = consts.tile([4, D + 1], f32)
        for k in range(4):
            nc.gpsimd.dma_start(
                out=cp4[k:k + 1, 0:D], in_=cpstage[32 * k:32 * k + 1, :]
            )
        # normalizer column: S_b lands at cp4[2b, D] (other rows zeroed)
        nc.gpsimd.memset(cp4[:, D:D + 1], 0.0)
        for b in range(BPC):
            nc.gpsimd.dma_start(
                out=cp4[2 * b:2 * b + 1, D:D + 1], in_=ssb[0:1, b:b + 1]
            )
        ps2 = pssc.tile([P, CH], f32, tag="pssc", name="ps2")
        nc.tensor.matmul(
            ps2[0:BPC, 0:D + 1], lhsT=sel4, rhs=cp4, start=True, stop=True
        )
        inv2 = consts.tile([BPC, 1], f32)
        nc.vector.reciprocal(out=inv2, in_=ps2[0:BPC, D:D + 1])
        ctxout = consts.tile([BPC, D], f32)
        nc.vector.tensor_scalar_mul(
            out=ctxout, in0=ps2[0:BPC, 0:D], scalar1=inv2
        )
        nc.sync.dma_start(out=out, in_=ctxout)

    nc.finalize()
    return nc


def _run(inputs, trace=False):
    from concourse import bass_utils

    nc = _build()
    in_maps = [
        {
            "query": np.ascontiguousarray(inputs["query"][BPC * i:BPC * (i + 1)]),
            "value": np.ascontiguousarray(inputs["value"][BPC * i:BPC * (i + 1)]),
            "W1": np.asarray(inputs["W1"]),
            "W2": np.asarray(inputs["W2"]),
            "V": np.asarray(inputs["V"]),
        }
        for i in range(NCORES)
    ]
    res = bass_utils.run_bass_kernel_spmd(
        nc, in_maps, core_ids=list(range(NCORES)), trace=trace
    )
    outp = np.concatenate([r["out"] for r in res.results], axis=0)
    return outp.astype(np.float32), res


def kernel(**inputs) -> np.ndarray:
    outp, _ = _run(inputs, trace=False)
    return outp


# revision 17
# speedup vs baseline: 1.1036x; 1.0579x over previous
"""Bahdanau additive attention on 8 Trainium2 NeuronCores.

  score_t = V^T tanh(W1 value_t + W2 query);  out = softmax(score) @ value

Sharding: data-parallel over batch (16 batches -> 2 per core), weights
replicated.

Design: a per-supertile (1024 t) software pipeline, all bf16 compute
against fp32 PSUM accumulation:
  load fp32 (8 KiB lines, p-major)                   [DMA, sync ring]
  -> cast bf16                                       [DVE]
  -> xbar DMA-transpose (value^T blocks)             [DMA, sync ring]
  -> keys^T = W1^T @ value^T per 512-chunk           [PE, 4-deep PSUM]
  -> tanh(psK + W2 q bias) per chunk                 [ACT]
  -> score rows = V^T th                             [PE]
  -> score row evacuation PSUM->SBUF                 [ACT/DVE]
  -> per-4-supertile-group scatter -> [32,128]       [DMA, sync ring]
  -> PE transpose -> exp -> e^T bf16                 [PE, ACT]
  -> context += e^T_f * value_f  (bf16)              [PE, streaming PSUM]
Stage lags keep the sync ring's descriptor generator (the xbar's real
serializing cost) fed with loads ahead of every transpose, and PSUM is
chunked [128,512] x4 so keys/tanh ping-pong never stalls the array.
"""

import functools
import os
import sys

import numpy as np

if "/opt/trn_rl_repo" not in sys.path:
    sys.path.insert(0, "/opt/trn_rl_repo")

B, T, D, U = 16, 8192, 256, 256
NCORES = 8
BPC = B // NCORES          # batches per core
P = 128                    # SBUF partitions
ST = 1024                  # t per supertile
NST = T // ST              # supertiles per batch
NSLOT = BPC * NST          # pipeline slots per core
CH = 512                   # score/keys chunk width (PSUM bank = 512 fp32)
NCH = ST // CH             # chunks per supertile
FPB = ST // P              # f rows per supertile (8)
GRP = 4                    # supertiles per score-transpose group


@functools.lru_cache(maxsize=1)
def _build():
    from contextlib import ExitStack

    import concourse.bass as bass
    import concourse.tile as tile
    from concourse import bacc, mybir
    from concourse.masks import make_identity

    f32 = mybir.dt.float32
    bf16 = mybir.dt.bfloat16
    Act = mybir.ActivationFunctionType

    nc = bacc.Bacc("TRN2", target_bir_lowering=False, debug=False)

    q = nc.dram_tensor("query", [BPC, D], f32, kind="ExternalInput").ap()
    val = nc.dram_tensor("value", [BPC, T, D], f32, kind="ExternalInput").ap()
    w1 = nc.dram_tensor("W1", [D, U], f32, kind="ExternalInput").ap()
    w2 = nc.dram_tensor("W2", [D, U], f32, kind="ExternalInput").ap()
    vv = nc.dram_tensor("V", [U, 1], f32, kind="ExternalInput").ap()
    out = nc.dram_tensor("out", [BPC, D], f32, kind="ExternalOutput").ap()

    with tile.TileContext(nc) as tc, ExitStack() as ctx:
        consts = ctx.enter_context(tc.tile_pool(name="consts", bufs=1))
        v32pool = ctx.enter_context(tc.tile_pool(name="v32", bufs=8))
        vbpool = ctx.enter_context(tc.tile_pool(name="vb", bufs=12))
        vtpool = ctx.enter_context(tc.tile_pool(name="vt", bufs=8))
        thpool = ctx.enter_context(tc.tile_pool(name="th", bufs=3))
        scpool = ctx.enter_context(tc.tile_pool(name="scrow", bufs=2))
        s8pool = ctx.enter_context(tc.tile_pool(name="s8", bufs=2))
        psk = ctx.enter_context(tc.tile_pool(name="psk", bufs=4, space="PSUM"))
        pssc = ctx.enter_context(tc.tile_pool(name="pssc", bufs=2, space="PSUM"))
        pst = ctx.enter_context(tc.tile_pool(name="pst", bufs=1, space="PSUM"))
        psc = ctx.enter_context(tc.tile_pool(name="psc", bufs=1, space="PSUM"))

        # ---- constants / weights (DMA on the gpsimd ring so value loads
        # own the sync ring from the first instruction) -------------------
        ident = consts.tile([64, 64], f32)
        make_identity(nc, ident)

        ones = consts.tile([P, 1], f32)
        nc.gpsimd.memset(ones, 1.0)

        # sel4 = [[1,0],[1,0],[0,1],[0,1]] — engine partition ranges must be
        # 32-aligned, so rows 2-3 are set via tiny DMAs from the identity.
        sel4 = consts.tile([4, 2], f32)
        nc.gpsimd.memset(sel4, 0.0)
        nc.gpsimd.memset(sel4[0:2, 0:1], 1.0)
        for r in (2, 3):
            nc.gpsimd.dma_start(out=sel4[r:r + 1, 1:2], in_=ident[0:1, 0:1])

        # W1 in d-half layout (d = kb*128 + p), bf16
        w1f = consts.tile([P, 2, U], f32)
        nc.gpsimd.dma_start(
            out=w1f, in_=w1.rearrange("(kb p) u -> p kb u", p=P)
        )
        w1b = consts.tile([P, 2, U], bf16)
        nc.vector.tensor_copy(out=w1b, in_=w1f)
        # V in u-half layout (u = ub*128 + p), bf16
        vvf = consts.tile([P, 2, 1], f32)
        nc.gpsimd.dma_start(out=vvf, in_=vv.rearrange("(ub p) o -> p ub o", p=P))
        vsb = consts.tile([P, 2, 1], bf16)
        nc.vector.tensor_copy(out=vsb, in_=vvf)

        w2b = consts.tile([P, 2, U], f32)
        nc.gpsimd.dma_start(out=w2b, in_=w2.rearrange("(kb p) u -> p kb u", p=P))

        # hidden = query @ W2, computed as hidden^T [u, b] so it can feed
        # the tanh as a per-partition bias.
        q_sb = consts.tile([BPC, D], f32)
        nc.gpsimd.dma_start(out=q_sb, in_=q)
        qt = consts.tile([P, 2, BPC], f32)
        for kb in range(2):
            psq = pssc.tile([P, CH], f32, tag="pssc", name="psq")
            nc.tensor.transpose(
                out=psq[:, 0:BPC],
                in_=q_sb[:, P * kb:P * (kb + 1)],
                identity=ident[0:BPC, 0:BPC],
            )
            nc.scalar.copy(out=qt[:, kb, :], in_=psq[:, 0:BPC])
        hid = []
        for u in range(2):
            psh = pssc.tile([P, CH], f32, tag="pssc", name="psh")
            for kb in range(2):
                nc.tensor.matmul(
                    psh[:, 0:BPC],
                    lhsT=w2b[:, kb, P * u:P * (u + 1)],
                    rhs=qt[:, kb, :],
                    start=(kb == 0),
                    stop=(kb == 1),
                )
            h = consts.tile([P, BPC], f32, tag=f"hid{u}")
            nc.scalar.copy(out=h, in_=psh[:, 0:BPC])
            hid.append(h)

        eT = [
            consts.tile([P, FPB * NST], bf16, tag=f"eT{b}", name=f"eT{b}")
            for b in range(BPC)
        ]

        # persistent context accumulator: rows {0,32} = batch 0 (f even/odd),
        # rows {64,96} = batch 1.
        psC = psc.tile([P, CH], f32, tag="psc", name="psc")

        vb32s, vbs, vts = {}, {}, {}
        scrows, s8s = {}, {}

        def emit_load(s):
            b, j = divmod(s, NST)
            VB32 = v32pool.tile([P, FPB, D], f32, tag="vb32", name="vb32")
            vb32s[s] = VB32
            nc.sync.dma_start(
                out=VB32,
                in_=val[b, ST * j:ST * (j + 1), :].rearrange(
                    "(p f) d -> p f d", f=FPB
                ),
            )

        def emit_casts(s):
            VB32 = vb32s[s]
            VB = vbpool.tile([P, 2, FPB, P], bf16, tag="vb", name="vb")
            vbs[s] = VB
            nc.vector.tensor_copy(
                out=VB, in_=VB32.rearrange("p f (h d) -> p h f d", h=2)
            )

        def emit_xbar(s):
            VB = vbs[s]
            VT = vtpool.tile([P, 2, FPB, P], bf16, tag="vt", name="vt")
            vts[s] = VT
            nc.sync.dma_start(
                out=VT.rearrange("p h f t -> p (h f) t"),
                in_=VB.rearrange("p h f t -> p (h f t)"),
                transpose=True,
            )

        def emit_mid(s):
            b, j = divmod(s, NST)
            VT = vts[s]
            th = thpool.tile([P, 2, ST], bf16, tag="th", name="th")
            for uh in range(2):
                psKc = []
                for c in range(NCH):
                    psKc.append(psk.tile([P, CH], f32, tag="psk", name="psk"))
                for dk in range(2):
                    for c in range(NCH):
                        nc.tensor.matmul(
                            psKc[c],
                            lhsT=w1b[:, dk, P * uh:P * (uh + 1)],
                            rhs=VT[:, dk, 4 * c:4 * (c + 1), :].rearrange(
                                "p f t -> p (f t)"
                            ),
                            start=(dk == 0),
                            stop=(dk == 1),
                        )
                for c in range(NCH):
                    nc.scalar.activation(
                        out=th[:, uh, CH * c:CH * (c + 1)],
                        in_=psKc[c],
                        func=Act.Tanh,
                        bias=hid[uh][:, b:b + 1],
                        scale=1.0,
                    )
            psSC = pssc.tile([P, CH], f32, tag="pssc", name="pssc")
            for c in range(NCH):
                for uh in range(2):
                    nc.tensor.matmul(
                        psSC[32 * c:32 * c + 1, :],
                        lhsT=vsb[:, uh, :],
                        rhs=th[:, uh, CH * c:CH * (c + 1)],
                        start=(uh == 0),
                        stop=(uh == 1),
                        tile_position=(0, 32 * c),
                    )
            g, r = divmod(s, GRP)
            if r == 0:
                scrow = scpool.tile(
                    [P, GRP * CH], f32, tag="scrow", name="scrow"
                )
                scrows[g] = scrow
            scrow = scrows[g]
            nc.scalar.copy(
                out=scrow[0:1, CH * r:CH * (r + 1)], in_=psSC[0:1, :]
            )
            nc.vector.tensor_copy(
                out=scrow[32:33, CH * r:CH * (r + 1)], in_=psSC[32:33, :]
            )

        def emit_scatter(g):
            scrow = scrows[g]
            s8 = s8pool.tile([GRP * FPB, P], f32, tag="s8", name="s8")
            s8s[g] = s8
            for r in range(GRP):
                for c in range(NCH):
                    row = FPB * r + 4 * c
                    nc.gpsimd.dma_start(
                        out=s8[row:row + 4, :],
                        in_=scrow[
                            32 * c:32 * c + 1, CH * r:CH * (r + 1)
                        ].rearrange("o (k f) -> o k f", k=4),
                    )

        def emit_tail(g):
            # handles slots GRP*g .. GRP*g+3 (all within one batch)
            s8 = s8s[g]
            psT = pst.tile([P, GRP * FPB], f32, tag="pst", name="pst")
            nc.tensor.transpose(
                out=psT, in_=s8, identity=ident[0:GRP * FPB, 0:GRP * FPB]
            )
            b = (GRP * g) // NST
            gj = (GRP * g) % NST          # first supertile of the group
            nc.scalar.activation(
                out=eT[b][:, FPB * gj:FPB * (gj + GRP)],
                in_=psT,
                func=Act.Exp,
                scale=1.0,
            )
            for r in range(GRP):
                s = GRP * g + r
                j = gj + r
                VB = vbs[s]
                for f in range(FPB):
                    par = f % 2
                    row = 32 * (2 * b + par)
                    nc.tensor.matmul(
                        psC[row:row + 1, 0:D],
                        lhsT=eT[b][:, FPB * j + f:FPB * j + f + 1],
                        rhs=VB[:, :, f, :],
                        start=(j == 0 and f == par),
                        stop=(j == NST - 1 and f == FPB - 2 + par),
                        tile_position=(0, row),
                    )

        for s in range(NSLOT + 8):
            if s < NSLOT:
                emit_load(s)
            if 0 <= s - 1 < NSLOT:
                emit_casts(s - 1)
            if 0 <= s - 2 < NSLOT:
                emit_xbar(s - 2)
            if 0 <= s - 4 < NSLOT:
                emit_mid(s - 4)
            if s >= 6 and (s - 6) % GRP == GRP - 1 and (s - 6) < NSLOT:
                emit_scatter((s - 6) // GRP)
            if s >= 7 and (s - 7) % GRP == GRP - 1 and (s - 7) < NSLOT:
                emit_tail((s - 7) // GRP)

        # ---- final combine: fold parity rows, normalize, store ----------
        # softmax normalizer S_b = sum(e): eT free-reduce then a ones-matmul
        # into one PSUM row; evacuated next to the context rows so the sel4
        # fold produces [ctx_b | S_b] per batch in one matmul.
        psS = pst.tile([P, GRP * FPB], f32, tag="pst", name="psS")
        for b in range(BPC):
            esum = consts.tile([P, 1], f32, tag=f"esum{b}", name=f"esum{b}")
            nc.vector.reduce_sum(out=esum, in_=eT[b], axis=mybir.AxisListType.X)
            nc.tensor.matmul(
                psS[0:1, b:b + 1], lhsT=ones, rhs=esum, start=True, stop=True
            )
        ssb = consts.tile([1, BPC], f32)
        nc.scalar.copy(out=ssb, in_=psS[0:1, 0:BPC])
        cpstage = consts.tile([P, D], f32)
        for r in (0, 32, 64, 96):
            nc.scalar.copy(out=cpstage[r:r + 1, :], in_=psC[r:r + 1, 0:D])
        cp4 = consts.tile([4, D + 1], f32)
        for k in range(4):
            nc.gpsimd.dma_start(
                out=cp4[k:k + 1, 0:D], in_=cpstage[32 * k:32 * k + 1, :]
            )
        # normalizer column: S_b lands at cp4[2b, D] (other rows zeroed)
        nc.gpsimd.memset(cp4[:, D:D + 1], 0.0)
        for b in range(BPC):
            nc.gpsimd.dma_start(
                out=cp4[2 * b:2 * b + 1, D:D + 1], in_=ssb[0:1, b:b + 1]
            )
        ps2 = pssc.tile([P, CH], f32, tag="pssc", name="ps2")
        nc.tensor.matmul(
            ps2[0:BPC, 0:D + 1], lhsT=sel4, rhs=cp4, start=True, stop=True
        )
        inv2 = consts.tile([BPC, 1], f32)
        nc.vector.reciprocal(out=inv2, in_=ps2[0:BPC, D:D + 1])
        ctxout = consts.tile([BPC, D], f32)
        nc.vector.tensor_scalar_mul(
            out=ctxout, in0=ps2[0:BPC, 0:D], scalar1=inv2
        )
        nc.sync.dma_start(out=out, in_=ctxout)

    nc.finalize()
    return nc


def _run(inputs, trace=False):
    from concourse import bass_utils

    nc = _build()
    in_maps = [
        {
            "query": np.ascontiguousarray(inputs["query"][BPC * i:BPC * (i + 1)]),
            "value": np.ascontiguousarray(inputs["value"][BPC * i:BPC * (i + 1)]),
            "W1": np.asarray(inputs["W1"]),
            "W2": np.asarray(inputs["W2"]),
            "V": np.asarray(inputs["V"]),
        }
        for i in range(NCORES)
    ]
    res = bass_utils.run_bass_kernel_spmd(
        nc, in_maps, core_ids=list(range(NCORES)), trace=trace
    )
    outp = np.concatenate([r["out"] for r in res.results], axis=0)
    return outp.astype(np.float32), res


def kernel(**inputs) -> np.ndarray:
    outp, _ = _run(inputs, trace=False)
    return outp
